# revision 3
# baseline (speedup 1.0000x reference)
"""CSPN (7x7 per-pixel spatial propagation) Trainium2 kernel.

Problem: out[b,0,y,x] = sum_{i,j in 0..6} gw[b, 7i+j, y+3, x+3] * src(y+3-i, x+3-j)
where src = hn (zero-padded outside [0,512)) except the center tap (i=j=3)
which uses h0. Shapes: gw [8,49,518,518] f32, hn/h0 [8,1,512,512] f32.

Strategy: pure data parallel - one batch element per NeuronCore (8 cores).
Per core, the 512x512 image is laid out as [128 partitions, 4 row-blocks,
512 cols]. The guide-weight read window is the same for every tap
(rows/cols 3:515), so each tap is one ~1MB DMA. The per-tap source shift
is absorbed by a pre-built zero-padded halo tensor S[p, k, b, u] =
hn[128b+p+k-3, u-3] in bf16, built via SBUF->SBUF DMAs (engines cannot
partition-shift). Two copies (S_even/S_odd, one element apart) keep the
bf16 tensor_tensor reads 4B-aligned for every tap so the DVE 2x mode
engages. Products and the 49-term accumulation run in bf16 on the DVE
(f32 weights are cast to bf16 on the Scalar engine, which is otherwise
idle); final result is cast back to f32 and stored.
"""

import numpy as np

_CACHE = {}


def _build_nc():
    import concourse.bacc as bacc
    import concourse.mybir as mybir
    import concourse.tile as tile

    F32 = mybir.dt.float32
    BF16 = mybir.dt.bfloat16
    MULT = mybir.AluOpType.mult
    ADD = mybir.AluOpType.add

    nc = bacc.Bacc("TRN2", target_bir_lowering=False, debug=False, num_devices=8)
    gw = nc.dram_tensor("gw", [49, 518, 518], F32, kind="ExternalInput").ap()
    hn = nc.dram_tensor("hn", [512, 512], F32, kind="ExternalInput").ap()
    h0 = nc.dram_tensor("h0", [512, 512], F32, kind="ExternalInput").ap()
    out = nc.dram_tensor("out", [512, 512], F32, kind="ExternalOutput").ap()

    with tile.TileContext(nc) as tc:
        with (
            tc.tile_pool(name="persist", bufs=1) as pp,
            tc.tile_pool(name="wf", bufs=4) as wfp,
            tc.tile_pool(name="wb", bufs=4) as wbp,
            tc.tile_pool(name="prod", bufs=3) as prp,
        ):
            # Stage hn/h0 as [p, b, x] and cast to bf16.
            hnf = pp.tile([128, 4, 512], F32)
            nc.sync.dma_start(out=hnf[:], in_=hn.rearrange("(b p) x -> p b x", p=128))
            hnb = pp.tile([128, 4, 512], BF16)
            nc.vector.tensor_copy(hnb[:], hnf[:])
            h0f = pp.tile([128, 4, 512], F32)
            nc.sync.dma_start(out=h0f[:], in_=h0.rearrange("(b p) x -> p b x", p=128))
            h0b = pp.tile([128, 4, 512], BF16)
            nc.vector.tensor_copy(h0b[:], h0f[:])

            # Halo tensors: S[par][p, k, b, u] = hn[128b+p+k-3, u-3-par], zero
            # outside. par=1 copy is offset one element so odd-j taps read
            # 4B-aligned.
            S = []
            for par in range(2):
                s = pp.tile([128, 7, 4, 520], BF16, tag=f"s{par}")
                nc.vector.memset(s[:], 0.0)
                S.append(s)
            for k in range(7):
                d = k - 3
                for par, s in enumerate(S):
                    u0 = 3 + par
                    if d == 0:
                        nc.sync.dma_start(out=s[:, k, :, u0 : u0 + 512], in_=hnb[:])
                    elif d > 0:
                        nc.sync.dma_start(
                            out=s[0 : 128 - d, k, 0:4, u0 : u0 + 512],
                            in_=hnb[d:128, 0:4, :],
                        )
                        nc.sync.dma_start(
                            out=s[128 - d : 128, k, 0:3, u0 : u0 + 512],
                            in_=hnb[0:d, 1:4, :],
                        )
                    else:
                        nc.sync.dma_start(
                            out=s[-d:128, k, 0:4, u0 : u0 + 512],
                            in_=hnb[0 : 128 + d, 0:4, :],
                        )
                        nc.sync.dma_start(
                            out=s[0:-d, k, 1:4, u0 : u0 + 512],
                            in_=hnb[128 + d : 128, 0:3, :],
                        )

            # Main loop: stream one 512x512 weight window per tap, multiply
            # with the (shifted) source, accumulate in bf16.
            acc = pp.tile([128, 4, 512], BF16)
            for t in range(49):
                i, j = t // 7, t % 7
                wf = wfp.tile([128, 4, 512], F32, tag="wf")
                nc.sync.dma_start(
                    out=wf[:],
                    in_=gw[t, 3:515, 3:515].rearrange("(b p) x -> p b x", p=128),
                )
                wb = wbp.tile([128, 4, 512], BF16, tag="wb")
                nc.scalar.copy(out=wb[:], in_=wf[:])
                if t == 24:
                    src_ap = h0b[:]
                else:
                    par = j & 1
                    u0 = (6 - j) + par
                    src_ap = S[par][:, 6 - i, :, u0 : u0 + 512]
                if t == 0:
                    nc.vector.tensor_tensor(out=acc[:], in0=wb[:], in1=src_ap, op=MULT)
                else:
                    prod = prp.tile([128, 4, 512], BF16, tag="prod")
                    nc.vector.tensor_tensor(out=prod[:], in0=wb[:], in1=src_ap, op=MULT)
                    nc.vector.tensor_tensor(out=acc[:], in0=acc[:], in1=prod[:], op=ADD)

            outf = pp.tile([128, 4, 512], F32)
            nc.scalar.copy(out=outf[:], in_=acc[:])
            nc.sync.dma_start(
                out=out.rearrange("(b p) x -> p b x", p=128), in_=outf[:]
            )

    nc.compile()
    return nc


def get_nc():
    if "nc" not in _CACHE:
        _CACHE["nc"] = _build_nc()
    return _CACHE["nc"]


def kernel(guide_weight, hn, h0):
    from concourse.bass_utils import run_bass_kernel_spmd

    nc = get_nc()
    in_maps = [
        {
            "gw": np.ascontiguousarray(guide_weight[b], dtype=np.float32),
            "hn": np.ascontiguousarray(hn[b, 0], dtype=np.float32),
            "h0": np.ascontiguousarray(h0[b, 0], dtype=np.float32),
        }
        for b in range(8)
    ]
    res = run_bass_kernel_spmd(nc, in_maps, core_ids=list(range(8)))
    return np.stack([res.results[b]["out"] for b in range(8)])[:, None].astype(
        np.float32
    )


# revision 4
# speedup vs baseline: 1.0941x; 1.0941x over previous
"""CSPN (7x7 per-pixel spatial propagation) Trainium2 kernel.

Problem: out[b,0,y,x] = sum_{i,j in 0..6} gw[b, 7i+j, y+3, x+3] * src(y+3-i, x+3-j)
where src = hn (zero-padded outside [0,512)) except the center tap (i=j=3)
which uses h0. Shapes: gw [8,49,518,518] f32, hn/h0 [8,1,512,512] f32.

Strategy: pure data parallel - one batch element per NeuronCore (8 cores).
Per core, the 512x512 image is laid out as [128 partitions, 4 row-blocks,
512 cols]. The guide-weight read window is identical for every tap
(rows/cols 3:515), so each tap is one ~1MB DMA; that 51.4MB/core stream is
the memory-roofline term. The per-tap source shift is absorbed by a
zero-padded halo tensor S[p, k, b, u] = hn[128b+p+k-3, u-3] in bf16, built
via SBUF->SBUF DMAs (engine ops cannot start at unaligned partitions). A
second copy offset by one element (S_odd, built on the Scalar engine -
same-partition shift) keeps bf16 tensor_tensor reads 4B-aligned for odd-j
taps so the DVE 2x perf mode engages everywhere. Products and the 49-term
accumulation run in bf16 on the DVE; f32 weights are cast to bf16 on the
otherwise-idle Scalar engine. Taps are ordered so compute starts as soon
as the first halo plane is ready.
"""

import numpy as np

_CACHE = {}


def _build_nc():
    import concourse.bacc as bacc
    import concourse.mybir as mybir
    import concourse.tile as tile

    F32 = mybir.dt.float32
    BF16 = mybir.dt.bfloat16
    MULT = mybir.AluOpType.mult
    ADD = mybir.AluOpType.add

    nc = bacc.Bacc("TRN2", target_bir_lowering=False, debug=False, num_devices=8)
    gw = nc.dram_tensor("gw", [49, 518, 518], F32, kind="ExternalInput").ap()
    hn = nc.dram_tensor("hn", [512, 512], F32, kind="ExternalInput").ap()
    h0 = nc.dram_tensor("h0", [512, 512], F32, kind="ExternalInput").ap()
    out = nc.dram_tensor("out", [512, 512], F32, kind="ExternalOutput").ap()

    with tile.TileContext(nc) as tc:
        with (
            tc.tile_pool(name="persist", bufs=1) as pp,
            tc.tile_pool(name="wf", bufs=4) as wfp,
            tc.tile_pool(name="wb", bufs=4) as wbp,
            tc.tile_pool(name="prod", bufs=3) as prp,
        ):
            # Stage hn/h0 as [p, b, x] and cast to bf16.
            hnf = pp.tile([128, 4, 512], F32)
            nc.sync.dma_start(out=hnf[:], in_=hn.rearrange("(b p) x -> p b x", p=128))
            hnb = pp.tile([128, 4, 512], BF16)
            nc.vector.tensor_copy(hnb[:], hnf[:])
            h0f = pp.tile([128, 4, 512], F32)
            nc.sync.dma_start(out=h0f[:], in_=h0.rearrange("(b p) x -> p b x", p=128))
            h0b = pp.tile([128, 4, 512], BF16)
            nc.vector.tensor_copy(h0b[:], h0f[:])

            # Halo tensors: s0[p, k, b, u] = hn[128b+p+k-3, u-3] (zero outside
            # the image), s1 the same data one u-slot later so odd-j taps read
            # 4B-aligned. s0 planes are filled by SBUF->SBUF DMA from hnb;
            # only pad columns and row-gap slots need explicit zeros (engine
            # memsets must start at 32-aligned partitions, so row gaps clear
            # a full 32-partition stripe before the data DMA overwrites most
            # of it). s1 = per-plane Scalar-engine copy of s0 shifted by one
            # element; its u=0 column is never read, and the copied range
            # carries s0's zero padding.
            s0 = pp.tile([128, 7, 4, 520], BF16, tag="s0")
            s1 = pp.tile([128, 7, 4, 520], BF16, tag="s1")
            nc.vector.memset(s0[:, :, :, 0:3], 0.0)
            nc.vector.memset(s0[:, :, :, 515:520], 0.0)
            # Build planes in the tap processing order (k = 6 down to 0).
            for k in range(6, -1, -1):
                d = k - 3
                if d == 0:
                    nc.sync.dma_start(out=s0[:, k, :, 3:515], in_=hnb[:])
                elif d > 0:
                    nc.vector.memset(s0[96:128, k, 3, :], 0.0)
                    nc.sync.dma_start(
                        out=s0[0 : 128 - d, k, 0:4, 3:515],
                        in_=hnb[d:128, 0:4, :],
                    )
                    nc.sync.dma_start(
                        out=s0[128 - d : 128, k, 0:3, 3:515],
                        in_=hnb[0:d, 1:4, :],
                    )
                else:
                    nc.vector.memset(s0[0:32, k, 0, :], 0.0)
                    nc.sync.dma_start(
                        out=s0[-d:128, k, 0:4, 3:515],
                        in_=hnb[0 : 128 + d, 0:4, :],
                    )
                    nc.sync.dma_start(
                        out=s0[0:-d, k, 1:4, 3:515],
                        in_=hnb[128 + d : 128, 0:3, :],
                    )
                nc.scalar.copy(out=s1[:, k, :, 1:520], in_=s0[:, k, :, 0:519])

            # Main loop: stream one 512x512 weight window per tap, multiply
            # with the (shifted) source, accumulate in bf16. Taps ordered by
            # halo-plane availability (i ascending => k=6-i descending), even
            # j (s0) before odd j (s1).
            order = []
            for i in range(7):
                order += [7 * i + j for j in (0, 2, 4, 6)]
                order += [7 * i + j for j in (1, 3, 5)]
            acc = pp.tile([128, 4, 512], BF16)
            for n, t in enumerate(order):
                i, j = t // 7, t % 7
                wf = wfp.tile([128, 4, 512], F32, tag="wf")
                nc.sync.dma_start(
                    out=wf[:],
                    in_=gw[t, 3:515, 3:515].rearrange("(b p) x -> p b x", p=128),
                )
                wb = wbp.tile([128, 4, 512], BF16, tag="wb")
                nc.scalar.copy(out=wb[:], in_=wf[:])
                if t == 24:
                    src_ap = h0b[:]
                elif j % 2 == 0:
                    src_ap = s0[:, 6 - i, :, 6 - j : 518 - j]
                else:
                    src_ap = s1[:, 6 - i, :, 7 - j : 519 - j]
                if n == 0:
                    nc.vector.tensor_tensor(out=acc[:], in0=wb[:], in1=src_ap, op=MULT)
                else:
                    prod = prp.tile([128, 4, 512], BF16, tag="prod")
                    nc.vector.tensor_tensor(out=prod[:], in0=wb[:], in1=src_ap, op=MULT)
                    nc.vector.tensor_tensor(out=acc[:], in0=acc[:], in1=prod[:], op=ADD)

            outf = pp.tile([128, 4, 512], F32)
            nc.scalar.copy(out=outf[:], in_=acc[:])
            nc.sync.dma_start(
                out=out.rearrange("(b p) x -> p b x", p=128), in_=outf[:]
            )

    nc.compile()
    return nc


def get_nc():
    if "nc" not in _CACHE:
        _CACHE["nc"] = _build_nc()
    return _CACHE["nc"]


def kernel(guide_weight, hn, h0):
    from concourse.bass_utils import run_bass_kernel_spmd

    nc = get_nc()
    in_maps = [
        {
            "gw": np.ascontiguousarray(guide_weight[b], dtype=np.float32),
            "hn": np.ascontiguousarray(hn[b, 0], dtype=np.float32),
            "h0": np.ascontiguousarray(h0[b, 0], dtype=np.float32),
        }
        for b in range(8)
    ]
    res = run_bass_kernel_spmd(nc, in_maps, core_ids=list(range(8)))
    return np.stack([res.results[b]["out"] for b in range(8)])[:, None].astype(
        np.float32
    )


# revision 6
# speedup vs baseline: 1.1057x; 1.0106x over previous
"""CSPN (7x7 per-pixel spatial propagation) Trainium2 kernel.

Problem: out[b,0,y,x] = sum_{i,j in 0..6} gw[b, 7i+j, y+3, x+3] * src(y+3-i, x+3-j)
where src = hn (zero-padded outside [0,512)) except the center tap (i=j=3)
which uses h0. Shapes: gw [8,49,518,518] f32, hn/h0 [8,1,512,512] f32.

Strategy: pure data parallel - one batch element per NeuronCore (8 cores).
Per core, the 512x512 image is laid out as [128 partitions, 4 row-blocks,
512 cols]. The guide-weight read window is identical for every tap
(rows/cols 3:515), so each tap is one ~1MB DMA; that 51.4MB/core stream is
the memory-roofline term. The per-tap source shift is absorbed by a
zero-padded halo tensor S[p, k, b, u] = hn[128b+p+k-3, u-3] in bf16, built
via SBUF->SBUF DMAs (engine ops cannot start at unaligned partitions). A
second copy offset by one element (S_odd, built on the Scalar engine -
same-partition shift) keeps bf16 tensor_tensor reads 4B-aligned for odd-j
taps so the DVE 2x perf mode engages everywhere. Products and the 49-term
accumulation run in bf16 on the DVE; f32 weights are cast to bf16 on the
otherwise-idle Scalar engine. Taps are ordered so compute starts as soon
as the first halo plane is ready.
"""

import numpy as np

_CACHE = {}


def _build_nc():
    import concourse.bacc as bacc
    import concourse.mybir as mybir
    import concourse.tile as tile

    F32 = mybir.dt.float32
    BF16 = mybir.dt.bfloat16
    MULT = mybir.AluOpType.mult
    ADD = mybir.AluOpType.add

    nc = bacc.Bacc("TRN2", target_bir_lowering=False, debug=False, num_devices=8)
    gw = nc.dram_tensor("gw", [49, 518, 518], F32, kind="ExternalInput").ap()
    hn = nc.dram_tensor("hn", [512, 512], F32, kind="ExternalInput").ap()
    h0 = nc.dram_tensor("h0", [512, 512], F32, kind="ExternalInput").ap()
    out = nc.dram_tensor("out", [512, 512], F32, kind="ExternalOutput").ap()

    with tile.TileContext(nc) as tc:
        with (
            tc.tile_pool(name="persist", bufs=1) as pp,
            tc.tile_pool(name="wf", bufs=4) as wfp,
            tc.tile_pool(name="wb", bufs=4) as wbp,
            tc.tile_pool(name="prod", bufs=3) as prp,
        ):
            # Stage hn/h0 as [p, b, x] and cast to bf16.
            hnf = pp.tile([128, 4, 512], F32)
            nc.sync.dma_start(out=hnf[:], in_=hn.rearrange("(b p) x -> p b x", p=128))
            hnb = pp.tile([128, 4, 512], BF16)
            nc.vector.tensor_copy(hnb[:], hnf[:])
            h0f = pp.tile([128, 4, 512], F32)
            nc.sync.dma_start(out=h0f[:], in_=h0.rearrange("(b p) x -> p b x", p=128))
            h0b = pp.tile([128, 4, 512], BF16)
            nc.vector.tensor_copy(h0b[:], h0f[:])

            # Halo tensors: s0[p, k, b, u] = hn[128b+p+k-3, u-3] (zero outside
            # the image), s1 the same data one u-slot later so odd-j taps read
            # 4B-aligned. s0 planes are filled by SBUF->SBUF DMA from hnb;
            # only pad columns and row-gap slots need explicit zeros (engine
            # memsets must start at 32-aligned partitions, so row gaps clear
            # a full 32-partition stripe before the data DMA overwrites most
            # of it). s1 = per-plane Scalar-engine copy of s0 shifted by one
            # element; its u=0 column is never read, and the copied range
            # carries s0's zero padding.
            s0 = pp.tile([128, 7, 4, 520], BF16, tag="s0")
            s1 = pp.tile([128, 7, 4, 520], BF16, tag="s1")
            nc.vector.memset(s0[:, :, :, 0:3], 0.0)
            nc.vector.memset(s0[:, :, :, 515:520], 0.0)
            # Build planes in the tap processing order (k = 6 down to 0).
            for k in range(6, -1, -1):
                d = k - 3
                if d == 0:
                    nc.sync.dma_start(out=s0[:, k, :, 3:515], in_=hnb[:])
                elif d > 0:
                    nc.vector.memset(s0[96:128, k, 3, :], 0.0)
                    nc.sync.dma_start(
                        out=s0[0 : 128 - d, k, 0:4, 3:515],
                        in_=hnb[d:128, 0:4, :],
                    )
                    nc.sync.dma_start(
                        out=s0[128 - d : 128, k, 0:3, 3:515],
                        in_=hnb[0:d, 1:4, :],
                    )
                else:
                    nc.vector.memset(s0[0:32, k, 0, :], 0.0)
                    nc.sync.dma_start(
                        out=s0[-d:128, k, 0:4, 3:515],
                        in_=hnb[0 : 128 + d, 0:4, :],
                    )
                    nc.sync.dma_start(
                        out=s0[0:-d, k, 1:4, 3:515],
                        in_=hnb[128 + d : 128, 0:3, :],
                    )
                nc.scalar.copy(out=s1[:, k, :, 1:520], in_=s0[:, k, :, 0:519])

            # Main loop: stream one 512x512 weight window per tap, multiply
            # with the (shifted) source, accumulate in bf16. Taps ordered by
            # halo-plane availability (i ascending => k=6-i descending), even
            # j (s0) before odd j (s1).
            order = []
            for i in range(7):
                order += [7 * i + j for j in (0, 2, 4, 6)]
                order += [7 * i + j for j in (1, 3, 5)]
            acc = pp.tile([128, 4, 512], BF16)
            outf = pp.tile([128, 4, 512], F32)
            out_ap = out.rearrange("(b p) x -> p b x", p=128)
            ntaps = len(order)
            for n, t in enumerate(order):
                i, j = t // 7, t % 7
                wf = wfp.tile([128, 4, 512], F32, tag="wf")
                nc.sync.dma_start(
                    out=wf[:],
                    in_=gw[t, 3:515, 3:515].rearrange("(b p) x -> p b x", p=128),
                )
                if t == 24:
                    src_ap = h0b[:]
                elif j % 2 == 0:
                    src_ap = s0[:, 6 - i, :, 6 - j : 518 - j]
                else:
                    src_ap = s1[:, 6 - i, :, 7 - j : 519 - j]
                if n < ntaps - 2:
                    # Steady state: cast the weights to bf16 on the Scalar
                    # engine so the DVE multiply runs in 2x mode.
                    wb = wbp.tile([128, 4, 512], BF16, tag="wb")
                    nc.scalar.copy(out=wb[:], in_=wf[:])
                    if n == 0:
                        nc.vector.tensor_tensor(
                            out=acc[:], in0=wb[:], in1=src_ap, op=MULT
                        )
                    else:
                        prod = prp.tile([128, 4, 512], BF16, tag="prod")
                        nc.vector.tensor_tensor(
                            out=prod[:], in0=wb[:], in1=src_ap, op=MULT
                        )
                        nc.vector.tensor_tensor(
                            out=acc[:], in0=acc[:], in1=prod[:], op=ADD
                        )
                elif n == ntaps - 2:
                    # Tail: skip the ACT cast hop (mixed f32xbf16 multiply) to
                    # shorten the dependency chain after the last weight DMAs.
                    prod = prp.tile([128, 4, 512], BF16, tag="prod")
                    nc.vector.tensor_tensor(out=prod[:], in0=wf[:], in1=src_ap, op=MULT)
                    nc.vector.tensor_tensor(out=acc[:], in0=acc[:], in1=prod[:], op=ADD)
                else:
                    # Last tap: per-block multiply/add/cast/store so the four
                    # output stores drain as a pipeline instead of waiting for
                    # the full-tile accumulation.
                    for b in range(4):
                        prod = prp.tile([128, 512], BF16, tag="prodb")
                        nc.vector.tensor_tensor(
                            out=prod[:],
                            in0=wf[:, b, :],
                            in1=src_ap[:, b, :] if t != 24 else h0b[:, b, :],
                            op=MULT,
                        )
                        nc.vector.tensor_tensor(
                            out=acc[:, b, :], in0=acc[:, b, :], in1=prod[:], op=ADD
                        )
                        nc.scalar.copy(out=outf[:, b, :], in_=acc[:, b, :])
                        nc.sync.dma_start(out=out_ap[:, b, :], in_=outf[:, b, :])

    nc.compile()
    return nc


def get_nc():
    if "nc" not in _CACHE:
        _CACHE["nc"] = _build_nc()
    return _CACHE["nc"]


def kernel(guide_weight, hn, h0):
    from concourse.bass_utils import run_bass_kernel_spmd

    nc = get_nc()
    in_maps = [
        {
            "gw": np.ascontiguousarray(guide_weight[b], dtype=np.float32),
            "hn": np.ascontiguousarray(hn[b, 0], dtype=np.float32),
            "h0": np.ascontiguousarray(h0[b, 0], dtype=np.float32),
        }
        for b in range(8)
    ]
    res = run_bass_kernel_spmd(nc, in_maps, core_ids=list(range(8)))
    return np.stack([res.results[b]["out"] for b in range(8)])[:, None].astype(
        np.float32
    )


# revision 8
# speedup vs baseline: 1.1067x; 1.0009x over previous
"""CSPN (7x7 per-pixel spatial propagation) Trainium2 kernel.

Problem: out[b,0,y,x] = sum_{i,j in 0..6} gw[b, 7i+j, y+3, x+3] * src(y+3-i, x+3-j)
where src = hn (zero-padded outside [0,512)) except the center tap (i=j=3)
which uses h0. Shapes: gw [8,49,518,518] f32, hn/h0 [8,1,512,512] f32.

Strategy: pure data parallel - one batch element per NeuronCore (8 cores).
Per core, the 512x512 image is laid out as [128 partitions, 4 row-blocks,
512 cols]. The guide-weight read window is identical for every tap
(rows/cols 3:515), so each tap is one ~1MB DMA; that 51.4MB/core stream is
the memory-roofline term. The per-tap source shift is absorbed by a
zero-padded halo tensor S[p, k, b, u] = hn[128b+p+k-3, u-3] in bf16, built
via SBUF->SBUF DMAs (engine ops cannot start at unaligned partitions). A
second copy offset by one element (S_odd, built on the Scalar engine -
same-partition shift) keeps bf16 tensor_tensor reads 4B-aligned for odd-j
taps so the DVE 2x perf mode engages everywhere. Products and the 49-term
accumulation run in bf16 on the DVE; f32 weights are cast to bf16 on the
otherwise-idle Scalar engine. Taps are ordered so compute starts as soon
as the first halo plane is ready.
"""

import numpy as np

_CACHE = {}


def _build_nc():
    import concourse.bacc as bacc
    import concourse.mybir as mybir
    import concourse.tile as tile

    F32 = mybir.dt.float32
    BF16 = mybir.dt.bfloat16
    MULT = mybir.AluOpType.mult
    ADD = mybir.AluOpType.add

    nc = bacc.Bacc("TRN2", target_bir_lowering=False, debug=False, num_devices=8)
    gw = nc.dram_tensor("gw", [49, 518, 518], F32, kind="ExternalInput").ap()
    hn = nc.dram_tensor("hn", [512, 512], F32, kind="ExternalInput").ap()
    h0 = nc.dram_tensor("h0", [512, 512], F32, kind="ExternalInput").ap()
    out = nc.dram_tensor("out", [512, 512], F32, kind="ExternalOutput").ap()

    with tile.TileContext(nc) as tc:
        with (
            tc.tile_pool(name="persist", bufs=1) as pp,
            tc.tile_pool(name="wf", bufs=4) as wfp,
            tc.tile_pool(name="wb", bufs=4) as wbp,
            tc.tile_pool(name="prod", bufs=3) as prp,
        ):
            # Stage hn/h0 as [p, b, x] and cast to bf16.
            hnf = pp.tile([128, 4, 512], F32)
            nc.sync.dma_start(out=hnf[:], in_=hn.rearrange("(b p) x -> p b x", p=128))
            hnb = pp.tile([128, 4, 512], BF16)
            nc.vector.tensor_copy(hnb[:], hnf[:])
            h0f = pp.tile([128, 4, 512], F32)
            nc.sync.dma_start(out=h0f[:], in_=h0.rearrange("(b p) x -> p b x", p=128))
            h0b = pp.tile([128, 4, 512], BF16)
            nc.vector.tensor_copy(h0b[:], h0f[:])

            # Halo tensors: s0[p, k, b, u] = hn[128b+p+k-3, u-3] (zero outside
            # the image), s1 the same data one u-slot later so odd-j taps read
            # 4B-aligned. s0 planes are filled by SBUF->SBUF DMA from hnb;
            # only pad columns and row-gap slots need explicit zeros (engine
            # memsets must start at 32-aligned partitions, so row gaps clear
            # a full 32-partition stripe before the data DMA overwrites most
            # of it). s1 = per-plane Scalar-engine copy of s0 shifted by one
            # element; its u=0 column is never read, and the copied range
            # carries s0's zero padding.
            s0 = pp.tile([128, 7, 4, 520], BF16, tag="s0")
            s1 = pp.tile([128, 7, 4, 520], BF16, tag="s1")
            nc.vector.memset(s0[:, :, :, 0:3], 0.0)
            nc.vector.memset(s0[:, :, :, 515:520], 0.0)
            # Build planes in the tap processing order (k = 6 down to 0).
            for k in range(6, -1, -1):
                d = k - 3
                if d == 0:
                    nc.sync.dma_start(out=s0[:, k, :, 3:515], in_=hnb[:])
                elif d > 0:
                    nc.vector.memset(s0[96:128, k, 3, :], 0.0)
                    nc.sync.dma_start(
                        out=s0[0 : 128 - d, k, 0:4, 3:515],
                        in_=hnb[d:128, 0:4, :],
                    )
                    nc.sync.dma_start(
                        out=s0[128 - d : 128, k, 0:3, 3:515],
                        in_=hnb[0:d, 1:4, :],
                    )
                else:
                    nc.vector.memset(s0[0:32, k, 0, :], 0.0)
                    nc.sync.dma_start(
                        out=s0[-d:128, k, 0:4, 3:515],
                        in_=hnb[0 : 128 + d, 0:4, :],
                    )
                    nc.sync.dma_start(
                        out=s0[0:-d, k, 1:4, 3:515],
                        in_=hnb[128 + d : 128, 0:3, :],
                    )
                nc.scalar.copy(out=s1[:, k, :, 1:520], in_=s0[:, k, :, 0:519])

            # Main loop: stream one 512x512 weight window per tap, multiply
            # with the (shifted) source, accumulate in bf16. Taps ordered by
            # halo-plane availability (i ascending => k=6-i descending), even
            # j (s0) before odd j (s1).
            order = []
            for i in range(7):
                order += [7 * i + j for j in (0, 2, 4, 6)]
                order += [7 * i + j for j in (1, 3, 5)]
            acc = pp.tile([128, 4, 512], BF16)
            outf = pp.tile([128, 4, 512], F32)
            out_ap = out.rearrange("(b p) x -> p b x", p=128)
            ntaps = len(order)
            for n, t in enumerate(order):
                i, j = t // 7, t % 7
                if n < ntaps - 1:
                    wf = wfp.tile([128, 4, 512], F32, tag="wf")
                    nc.sync.dma_start(
                        out=wf[:],
                        in_=gw[t, 3:515, 3:515].rearrange("(b p) x -> p b x", p=128),
                    )
                if t == 24:
                    src_ap = h0b[:]
                elif j % 2 == 0:
                    src_ap = s0[:, 6 - i, :, 6 - j : 518 - j]
                else:
                    src_ap = s1[:, 6 - i, :, 7 - j : 519 - j]
                if n < ntaps - 2:
                    # Steady state: cast the weights to bf16 on the Scalar
                    # engine so the DVE multiply runs in 2x mode.
                    wb = wbp.tile([128, 4, 512], BF16, tag="wb")
                    nc.scalar.copy(out=wb[:], in_=wf[:])
                    if n == 0:
                        nc.vector.tensor_tensor(
                            out=acc[:], in0=wb[:], in1=src_ap, op=MULT
                        )
                    else:
                        prod = prp.tile([128, 4, 512], BF16, tag="prod")
                        nc.vector.tensor_tensor(
                            out=prod[:], in0=wb[:], in1=src_ap, op=MULT
                        )
                        nc.vector.tensor_tensor(
                            out=acc[:], in0=acc[:], in1=prod[:], op=ADD
                        )
                elif n == ntaps - 2:
                    # Tail: skip the ACT cast hop (mixed f32xbf16 multiply) to
                    # shorten the dependency chain after the last weight DMAs.
                    prod = prp.tile([128, 4, 512], BF16, tag="prod")
                    nc.vector.tensor_tensor(out=prod[:], in0=wf[:], in1=src_ap, op=MULT)
                    nc.vector.tensor_tensor(out=acc[:], in0=acc[:], in1=prod[:], op=ADD)
                else:
                    # Last tap: per-block weight DMA / multiply / add / cast /
                    # store so the tail drains as a four-stage pipeline behind
                    # the final weight bytes instead of one full-tile chain.
                    for b in range(4):
                        wq = wfp.tile([128, 512], F32, tag="wq")
                        nc.sync.dma_start(
                            out=wq[:],
                            in_=gw[t, 3 + 128 * b : 131 + 128 * b, 3:515],
                        )
                        prod = prp.tile([128, 512], BF16, tag="prodb")
                        nc.vector.tensor_tensor(
                            out=prod[:], in0=wq[:], in1=src_ap[:, b, :], op=MULT
                        )
                        nc.vector.tensor_tensor(
                            out=acc[:, b, :], in0=acc[:, b, :], in1=prod[:], op=ADD
                        )
                        nc.scalar.copy(out=outf[:, b, :], in_=acc[:, b, :])
                        nc.sync.dma_start(out=out_ap[:, b, :], in_=outf[:, b, :])

    nc.compile()
    return nc


def get_nc():
    if "nc" not in _CACHE:
        _CACHE["nc"] = _build_nc()
    return _CACHE["nc"]


def kernel(guide_weight, hn, h0):
    from concourse.bass_utils import run_bass_kernel_spmd

    nc = get_nc()
    in_maps = [
        {
            "gw": np.ascontiguousarray(guide_weight[b], dtype=np.float32),
            "hn": np.ascontiguousarray(hn[b, 0], dtype=np.float32),
            "h0": np.ascontiguousarray(h0[b, 0], dtype=np.float32),
        }
        for b in range(8)
    ]
    res = run_bass_kernel_spmd(nc, in_maps, core_ids=list(range(8)))
    return np.stack([res.results[b]["out"] for b in range(8)])[:, None].astype(
        np.float32
    )


# revision 10
# speedup vs baseline: 1.1148x; 1.0074x over previous
"""CSPN (7x7 per-pixel spatial propagation) Trainium2 kernel.

Problem: out[b,0,y,x] = sum_{i,j in 0..6} gw[b, 7i+j, y+3, x+3] * src(y+3-i, x+3-j)
where src = hn (zero-padded outside [0,512)) except the center tap (i=j=3)
which uses h0. Shapes: gw [8,49,518,518] f32, hn/h0 [8,1,512,512] f32.

Strategy: pure data parallel - one batch element per NeuronCore (8 cores).
Per core, the 512x512 image is laid out as [128 partitions, 4 row-blocks,
512 cols]. The guide-weight read window is identical for every tap
(rows/cols 3:515), so each tap is one ~1MB DMA; that 51.4MB/core stream is
the memory-roofline term. The per-tap source shift is absorbed by a
zero-padded halo tensor S[p, k, b, u] = hn[128b+p+k-3, u-3] in bf16, built
via SBUF->SBUF DMAs (engine ops cannot start at unaligned partitions). A
second copy offset by one element (S_odd, built on the Scalar engine -
same-partition shift) keeps bf16 tensor_tensor reads 4B-aligned for odd-j
taps so the DVE 2x perf mode engages everywhere. Products and the 49-term
accumulation run in bf16 on the DVE; f32 weights are cast to bf16 on the
otherwise-idle Scalar engine. Taps are ordered so compute starts as soon
as the first halo plane is ready.
"""

import numpy as np

_CACHE = {}


def _build_nc():
    import concourse.bacc as bacc
    import concourse.mybir as mybir
    import concourse.tile as tile

    F32 = mybir.dt.float32
    BF16 = mybir.dt.bfloat16
    MULT = mybir.AluOpType.mult
    ADD = mybir.AluOpType.add

    nc = bacc.Bacc("TRN2", target_bir_lowering=False, debug=False, num_devices=8)
    gw = nc.dram_tensor("gw", [49, 518, 518], F32, kind="ExternalInput").ap()
    hn = nc.dram_tensor("hn", [512, 512], F32, kind="ExternalInput").ap()
    h0 = nc.dram_tensor("h0", [512, 512], F32, kind="ExternalInput").ap()
    out = nc.dram_tensor("out", [512, 512], F32, kind="ExternalOutput").ap()

    with tile.TileContext(nc) as tc:
        with (
            tc.tile_pool(name="persist", bufs=1) as pp,
            tc.tile_pool(name="wf", bufs=4) as wfp,
            tc.tile_pool(name="wb", bufs=4) as wbp,
            tc.tile_pool(name="prod", bufs=3) as prp,
        ):
            # Stage hn/h0 as [p, b, x] and cast to bf16.
            hnf = pp.tile([128, 4, 512], F32)
            nc.sync.dma_start(out=hnf[:], in_=hn.rearrange("(b p) x -> p b x", p=128))
            hnb = pp.tile([128, 4, 512], BF16)
            nc.vector.tensor_copy(hnb[:], hnf[:])
            h0f = pp.tile([128, 4, 512], F32)
            nc.sync.dma_start(out=h0f[:], in_=h0.rearrange("(b p) x -> p b x", p=128))
            h0b = pp.tile([128, 4, 512], BF16)
            nc.vector.tensor_copy(h0b[:], h0f[:])

            # Halo tensors: s0[p, k, b, u] = hn[128b+p+k-3, u-3] (zero outside
            # the image), s1 the same data one u-slot later so odd-j taps read
            # 4B-aligned. s0 planes are filled by SBUF->SBUF DMA from hnb;
            # only pad columns and row-gap slots need explicit zeros (engine
            # memsets must start at 32-aligned partitions, so row gaps clear
            # a full 32-partition stripe before the data DMA overwrites most
            # of it). s1 = per-plane Scalar-engine copy of s0 shifted by one
            # element; its u=0 column is never read, and the copied range
            # carries s0's zero padding.
            s0 = pp.tile([128, 7, 4, 520], BF16, tag="s0")
            s1 = pp.tile([128, 7, 4, 520], BF16, tag="s1")
            nc.vector.memset(s0[:, :, :, 0:3], 0.0)
            nc.vector.memset(s0[:, :, :, 515:520], 0.0)
            # Build planes in the tap processing order (k = 6 down to 0).
            for k in range(6, -1, -1):
                d = k - 3
                if d == 0:
                    nc.sync.dma_start(out=s0[:, k, :, 3:515], in_=hnb[:])
                elif d > 0:
                    nc.vector.memset(s0[96:128, k, 3, :], 0.0)
                    nc.sync.dma_start(
                        out=s0[0 : 128 - d, k, 0:4, 3:515],
                        in_=hnb[d:128, 0:4, :],
                    )
                    nc.sync.dma_start(
                        out=s0[128 - d : 128, k, 0:3, 3:515],
                        in_=hnb[0:d, 1:4, :],
                    )
                else:
                    nc.vector.memset(s0[0:32, k, 0, :], 0.0)
                    nc.sync.dma_start(
                        out=s0[-d:128, k, 0:4, 3:515],
                        in_=hnb[0 : 128 + d, 0:4, :],
                    )
                    nc.sync.dma_start(
                        out=s0[0:-d, k, 1:4, 3:515],
                        in_=hnb[128 + d : 128, 0:3, :],
                    )
                nc.scalar.copy(out=s1[:, k, :, 1:520], in_=s0[:, k, :, 0:519])

            # Main loop: stream one 512x512 weight window per tap, multiply
            # with the (shifted) source, accumulate in bf16. Taps ordered by
            # halo-plane availability (i ascending => k=6-i descending), even
            # j (s0) before odd j (s1).
            order = []
            for i in range(7):
                order += [7 * i + j for j in (0, 2, 4, 6)]
                order += [7 * i + j for j in (1, 3, 5)]
            acc = pp.tile([128, 4, 512], BF16)
            outf = pp.tile([128, 4, 512], F32)
            out_ap = out.rearrange("(b p) x -> p b x", p=128)
            ntaps = len(order)
            def src_for(t):
                i, j = t // 7, t % 7
                if t == 24:
                    return h0b[:]
                if j % 2 == 0:
                    return s0[:, 6 - i, :, 6 - j : 518 - j]
                return s1[:, 6 - i, :, 7 - j : 519 - j]

            for n, t in enumerate(order[: ntaps - 2]):
                wf = wfp.tile([128, 4, 512], F32, tag="wf")
                nc.sync.dma_start(
                    out=wf[:],
                    in_=gw[t, 3:515, 3:515].rearrange("(b p) x -> p b x", p=128),
                )
                # Cast the weights to bf16 on the Scalar engine so the DVE
                # multiply runs in 2x mode.
                wb = wbp.tile([128, 4, 512], BF16, tag="wb")
                nc.scalar.copy(out=wb[:], in_=wf[:])
                if n == 0:
                    nc.vector.tensor_tensor(out=acc[:], in0=wb[:], in1=src_for(t), op=MULT)
                else:
                    prod = prp.tile([128, 4, 512], BF16, tag="prod")
                    nc.vector.tensor_tensor(out=prod[:], in0=wb[:], in1=src_for(t), op=MULT)
                    nc.vector.tensor_tensor(out=acc[:], in0=acc[:], in1=prod[:], op=ADD)

            # Tail: process the last two taps block-striped (block 0's weight
            # quarters stream first) so each block's accumulate/cast/store
            # drains while later blocks' weights are still arriving.
            t_a, t_b = order[ntaps - 2], order[ntaps - 1]
            for b in range(4):
                for t in (t_a, t_b):
                    wq = wfp.tile([128, 512], F32, tag="wq")
                    nc.sync.dma_start(
                        out=wq[:], in_=gw[t, 3 + 128 * b : 131 + 128 * b, 3:515]
                    )
                    wbq = wbp.tile([128, 512], BF16, tag="wbq")
                    nc.scalar.copy(out=wbq[:], in_=wq[:])
                    prod = prp.tile([128, 512], BF16, tag="prodb")
                    nc.vector.tensor_tensor(
                        out=prod[:], in0=wbq[:], in1=src_for(t)[:, b, :], op=MULT
                    )
                    nc.vector.tensor_tensor(
                        out=acc[:, b, :], in0=acc[:, b, :], in1=prod[:], op=ADD
                    )
                nc.scalar.copy(out=outf[:, b, :], in_=acc[:, b, :])
                nc.sync.dma_start(out=out_ap[:, b, :], in_=outf[:, b, :])

    nc.compile()
    return nc


def get_nc():
    if "nc" not in _CACHE:
        _CACHE["nc"] = _build_nc()
    return _CACHE["nc"]


def kernel(guide_weight, hn, h0):
    from concourse.bass_utils import run_bass_kernel_spmd

    nc = get_nc()
    in_maps = [
        {
            "gw": np.ascontiguousarray(guide_weight[b], dtype=np.float32),
            "hn": np.ascontiguousarray(hn[b, 0], dtype=np.float32),
            "h0": np.ascontiguousarray(h0[b, 0], dtype=np.float32),
        }
        for b in range(8)
    ]
    res = run_bass_kernel_spmd(nc, in_maps, core_ids=list(range(8)))
    return np.stack([res.results[b]["out"] for b in range(8)])[:, None].astype(
        np.float32
    )
